# revision 21
# baseline (speedup 1.0000x reference)
"""Single-head causal attention on 8 Trainium2 NeuronCores.

B=4, T=4096, E=1024, H=128, fp32 in/out.

Sharding: batch-parallel x query-parallel. Two programs (one per query half):
  program A cores (devices 0-3): batch d, queries [0:1024) u [3072:4096)
  program B cores (devices 4-7): batch d-4, queries [1024:3072)
Both halves have identical causal work (72 key-tiles of 128) -> balanced.
Each core computes full K/V for its batch from x^T (host-transposed, fp16).

On-chip (per core); matmul operands fp16, accumulation fp32:
  1. QKV projections over 8 E-chunks accumulated in PSUM.
     K^T [H=128p, T] fp16, V^T fp16, Q^T [128p, 2048] fp16.
     V natural via PE transpose of V^T.
  2. Per 512-query tile, key tiles of 128 (diagonal emitted first):
     S^T[keys,q] = (K^T_kt).T @ Q^T_j -> PSUM fp32,
     E = exp(S^T/sqrt(H)) on ScalarE -> fp16 SBUF,
     causal mask on diagonal tiles via gpsimd affine_select,
     PV: OT[h,q] += V_kt.T @ E in PSUM; G0/G1 += E on DVE (fp16 2x mode).
  3. denom = colsum(G0)+colsum(G1) via ones-matmuls into PSUM, DVE
     reciprocal, PE-transpose OT chunks, row-scale by 1/denom, one
     gathered DMA per query tile -> out natural [2048,128] fp32.
"""

import numpy as np

import concourse.bass as bass
import concourse.bacc as bacc
import concourse.mybir as mybir
import concourse.tile as tile
from concourse.masks import make_identity

B, T, E, H = 4, 4096, 1024, 128
TQ = 512          # query tile width
NE = E // 128     # 8 e-chunks
NCC = T // TQ     # 8 column chunks of T
QROWS = 2048      # queries per core
SCALE = float(H) ** -0.5
F32 = mybir.dt.float32
F16 = mybir.dt.float16

T0S_A = [0, 512, 3072, 3584]
T0S_B = [1024, 1536, 2048, 2560]


def _build(t0s):
    nc = bacc.Bacc("TRN2", target_bir_lowering=False, debug=False, num_devices=4)
    xT = nc.declare_dram_parameter("xT", [E, T], F16, isOutput=False)
    Wq = nc.declare_dram_parameter("Wq", [E, H], F16, isOutput=False)
    Wk = nc.declare_dram_parameter("Wk", [E, H], F16, isOutput=False)
    Wv = nc.declare_dram_parameter("Wv", [E, H], F16, isOutput=False)
    out = nc.declare_dram_parameter("out", [QROWS, H], F32, isOutput=True)

    qcc = {t0 // TQ: j for j, t0 in enumerate(t0s)}  # T col-chunk -> q tile slot

    with tile.TileContext(nc) as tc:
        with (
            tc.tile_pool(name="const", bufs=1) as const_pool,
            tc.tile_pool(name="wts", bufs=1) as wt_pool,
            tc.tile_pool(name="big", bufs=1) as big_pool,
            tc.tile_pool(name="ev", bufs=4) as e_pool,
            tc.tile_pool(name="g", bufs=2) as g_pool,
            tc.tile_pool(name="ot", bufs=2) as ot_pool,
            tc.tile_pool(name="small", bufs=4) as small_pool,
            tc.tile_pool(name="onat", bufs=2) as onat_pool,
            tc.tile_pool(name="mm512", bufs=3, space="PSUM") as mm_psum,
            tc.tile_pool(name="pv", bufs=2, space="PSUM") as pv_psum,
            tc.tile_pool(name="tp", bufs=2, space="PSUM") as tp_psum,
            tc.tile_pool(name="dcol", bufs=1, space="PSUM") as dcol_psum,
        ):
            ident16 = const_pool.tile([128, 128], F16, tag="id16")
            make_identity(nc, ident16[:])
            ident32 = const_pool.tile([128, 128], F32, tag="id32")
            make_identity(nc, ident32[:])
            ones = const_pool.tile([128, 1], F16, tag="ones")
            nc.gpsimd.memset(ones[:], 1.0)

            # weights: [E,H] dram -> [128, NE*H] sbuf, e-slice at [:, e*H:(e+1)*H]
            wq_sb = wt_pool.tile([128, NE * H], F16, tag="wq")
            wk_sb = wt_pool.tile([128, NE * H], F16, tag="wk")
            wv_sb = wt_pool.tile([128, NE * H], F16, tag="wv")
            for w_dram, w_sb in ((Wq, wq_sb), (Wk, wk_sb), (Wv, wv_sb)):
                nc.sync.dma_start(
                    out=w_sb[:].rearrange("p (e h) -> p e h", e=NE),
                    in_=w_dram[:].rearrange("(e p) h -> p e h", p=128),
                )

            # x^T resident, loaded in consumption order across three issue
            # engines. Program B only ever touches keys/queries < kv_cols.
            kv_ccs = max((t0 + TQ) // TQ for t0 in t0s)       # 8 for A, 6 for B
            kv_cols = kv_ccs * TQ
            xts = []
            for e in range(NE):
                xt_t = big_pool.tile([128, kv_cols], F16, tag=f"xt{e}")
                xts.append(xt_t)
            for e in range(NE):         # cols 0:512 on sync: PE starts fast
                nc.sync.dma_start(
                    out=xts[e][:, 0:TQ],
                    in_=xT[e * 128:(e + 1) * 128, 0:TQ],
                )
            for e in range(NE):         # cols 512:1024 on scalar (idle early)
                nc.scalar.dma_start(
                    out=xts[e][:, TQ:1024],
                    in_=xT[e * 128:(e + 1) * 128, TQ:1024],
                )
            for q in range(1, 3):       # cols 1024:3072 on sync, [128,1024]
                for e in range(NE):
                    nc.sync.dma_start(
                        out=xts[e][:, q * 1024:(q + 1) * 1024],
                        in_=xT[e * 128:(e + 1) * 128, q * 1024:(q + 1) * 1024],
                    )
            if kv_cols > 3072:          # cols 3072:4096 on gpsimd (SWDGE)
                for e in range(NE):
                    nc.gpsimd.dma_start(
                        out=xts[e][:, 3072:kv_cols],
                        in_=xT[e * 128:(e + 1) * 128, 3072:kv_cols],
                    )

            KT = big_pool.tile([128, kv_cols], F16, tag="kt")   # K^T
            VT = big_pool.tile([128, kv_cols], F16, tag="vt")   # V^T
            V = big_pool.tile([128, kv_cols], F16, tag="v")     # V natural
            QT = big_pool.tile([128, QROWS], F16, tag="qt")     # Q^T

            def project(cc):
                c0 = cc * TQ
                for w_sb, dst, d0 in (
                    (wk_sb, KT, c0),
                    (wv_sb, VT, c0),
                ) + (((wq_sb, QT, qcc[cc] * TQ),) if cc in qcc else ()):
                    ps = mm_psum.tile([128, TQ], F32, tag="mm", name="ps")
                    for e in range(NE):
                        nc.tensor.matmul(
                            ps[:], w_sb[:, e * H:(e + 1) * H],
                            xts[e][:, c0:c0 + TQ],
                            start=(e == 0), stop=(e == NE - 1),
                        )
                    nc.vector.tensor_copy(dst[:, d0:d0 + TQ], ps[:])
                # V natural = transpose(V^T) for this chunk's key blocks
                for kt in range(cc * TQ // 128, (cc + 1) * TQ // 128):
                    tp16 = tp_psum.tile([128, 128], F16, tag="tp", name="tp16")
                    nc.tensor.transpose(
                        tp16[:], VT[:, kt * 128:(kt + 1) * 128], ident16[:]
                    )
                    nc.vector.tensor_copy(V[:, kt * 128:(kt + 1) * 128], tp16[:])

            def attend(j, t0):
                nkt = (t0 + TQ) // 128
                diag0 = t0 // 128
                ot_ps = pv_psum.tile([128, TQ], F32, name="ot_ps")
                G0 = g_pool.tile([128, TQ], F16, tag="g0", name="G0")
                G1 = g_pool.tile([128, TQ], F16, tag="g1", name="G1")
                kts = list(range(nkt - 1, -1, -1))  # diagonal first
                for i, kt in enumerate(kts):
                    st = mm_psum.tile([128, TQ], F32, tag="mm", name="st")
                    nc.tensor.matmul(
                        st[:], KT[:, kt * 128:(kt + 1) * 128],
                        QT[:, j * TQ:(j + 1) * TQ],
                        start=True, stop=True,
                    )
                    e_t = e_pool.tile([128, TQ], F16, name="e_t")
                    nc.scalar.activation(
                        e_t[:], st[:], mybir.ActivationFunctionType.Exp, scale=SCALE
                    )
                    if kt >= diag0:
                        # keep E[p,c] iff (t0+c) - (128*kt+p) >= 0
                        nc.gpsimd.affine_select(
                            out=e_t[:], in_=e_t[:],
                            compare_op=mybir.AluOpType.is_ge,
                            fill=0.0, base=t0 - 128 * kt,
                            pattern=[[1, TQ]], channel_multiplier=-1,
                        )
                    nc.tensor.matmul(
                        ot_ps[:], V[:, kt * 128:(kt + 1) * 128], e_t[:],
                        start=(i == 0), stop=(i == nkt - 1),
                    )
                    # exp-sum accumulation, two chains (halves dep depth)
                    if i < 2:
                        nc.vector.tensor_copy((G0 if i == 0 else G1)[:], e_t[:])
                    elif i % 2 == 0:
                        nc.vector.tensor_add(G0[:], G0[:], e_t[:])
                    else:
                        nc.vector.tensor_add(G1[:], G1[:], e_t[:])

                ot_sb = ot_pool.tile([128, TQ], F32, name="ot_sb")
                nc.scalar.copy(ot_sb[:], ot_ps[:])
                onat = onat_pool.tile([128, 4 * 128], F32, name="onat")
                single_g = nkt < 2
                for c in range(TQ // 128):
                    dps = dcol_psum.tile([128, 1], F32, name="dps")
                    nc.tensor.matmul(
                        dps[:], G0[:, c * 128:(c + 1) * 128], ones[:],
                        start=True, stop=single_g,
                    )
                    if not single_g:
                        nc.tensor.matmul(
                            dps[:], G1[:, c * 128:(c + 1) * 128], ones[:],
                            start=False, stop=True,
                        )
                    rc = small_pool.tile([128, 1], F32, name="rc")
                    nc.vector.reciprocal(rc[:], dps[:])
                    tp = tp_psum.tile([128, 128], F32, tag="tp", name="tp")
                    nc.tensor.transpose(
                        tp[:], ot_sb[:, c * 128:(c + 1) * 128], ident32[:]
                    )
                    nc.vector.tensor_scalar_mul(
                        onat[:, c * 128:(c + 1) * 128], tp[:], rc[:]
                    )
                # one DMA per query tile: [128, (c h)] -> out rows [j*TQ, +TQ)
                nc.sync.dma_start(
                    out=out[j * TQ:(j + 1) * TQ, :].rearrange(
                        "(c p) h -> p c h", p=128
                    ),
                    in_=onat[:].rearrange("p (c h) -> p c h", c=4),
                )

            # interleave: project each chunk, then run any attention tile
            # whose keys/queries are now fully projected
            done = set()
            for cc in range(kv_ccs):
                project(cc)
                for j, t0 in enumerate(t0s):
                    if j in done:
                        continue
                    if (t0 + TQ) // TQ <= cc + 1 and (t0 // TQ) <= cc:
                        done.add(j)
                        attend(j, t0)
            assert done == set(range(len(t0s)))

    nc.finalize()
    return nc


# ---------------- host-side run ----------------

_CACHE = {}


def _runner(nc, devices):
    """run_bass_via_pjrt with an explicit device list (subset launch)."""
    import jax
    from jax.sharding import Mesh, PartitionSpec
    from jax.experimental.shard_map import shard_map
    from concourse.bass2jax import _bass_exec_p, install_neuronx_cc_hook

    install_neuronx_cc_hook()
    n_cores = len(devices)
    part_name = nc.partition_id_tensor.name if nc.partition_id_tensor else None
    in_names, out_names, out_avals, zero_outs = [], [], [], []
    for alloc in nc.m.functions[0].allocations:
        if not isinstance(alloc, mybir.MemoryLocationSet):
            continue
        name = alloc.memorylocations[0].name
        if alloc.kind == "ExternalInput":
            if name != part_name:
                in_names.append(name)
        elif alloc.kind == "ExternalOutput":
            shape = tuple(alloc.tensor_shape)
            dtype = mybir.dt.np(alloc.dtype)
            out_names.append(name)
            out_avals.append(jax.core.ShapedArray(shape, dtype))
            zero_outs.append(np.zeros(shape, dtype))
    n_params = len(in_names)
    n_outs = len(out_avals)
    in_names = in_names + out_names
    if part_name is not None:
        in_names = in_names + [part_name]
    donate = tuple(range(n_params, n_params + n_outs))

    def _body(*args):
        from concourse.bass2jax import partition_id_tensor
        operands = list(args)
        if part_name is not None:
            operands.append(partition_id_tensor())
        outs = _bass_exec_p.bind(
            *operands,
            out_avals=tuple(out_avals),
            in_names=tuple(in_names),
            out_names=tuple(out_names),
            lowering_input_output_aliases=(),
            sim_require_finite=True,
            sim_require_nnan=True,
            nc=nc,
        )
        return tuple(outs)

    mesh = Mesh(np.asarray(devices), ("core",))
    sharded = jax.jit(
        shard_map(
            _body, mesh=mesh,
            in_specs=(PartitionSpec("core"),) * (n_params + n_outs),
            out_specs=(PartitionSpec("core"),) * n_outs,
            check_rep=False,
        ),
        donate_argnums=donate, keep_unused=True,
    )

    def run(in_maps):
        per_core = [[np.asarray(m[n]) for n in in_names[:n_params]] for m in in_maps]
        concat_in = [
            np.concatenate([per_core[c][i] for c in range(n_cores)], axis=0)
            for i in range(n_params)
        ]
        concat_zeros = [
            np.zeros((n_cores * z.shape[0], *z.shape[1:]), z.dtype) for z in zero_outs
        ]
        return sharded(*concat_in, *concat_zeros)

    def finish(out_arrs):
        return [
            {
                n: np.asarray(out_arrs[i]).reshape(n_cores, *out_avals[i].shape)[c]
                for i, n in enumerate(out_names)
            }
            for c in range(n_cores)
        ]

    return run, finish


def _get_runners():
    if "runners" not in _CACHE:
        import jax
        devs = jax.devices()
        ncA = _build(T0S_A)
        ncB = _build(T0S_B)
        _CACHE["ncs"] = (ncA, ncB)
        runA = _runner(ncA, devs[0:4])
        runB = _runner(ncB, devs[4:8])
        # Warm each executable once, sequentially and blocking, before
        # any concurrent use (cold concurrent dispatch has raced before).
        z = [
            {
                "xT": np.zeros((E, T), np.float16),
                "Wq": np.zeros((E, H), np.float16),
                "Wk": np.zeros((E, H), np.float16),
                "Wv": np.zeros((E, H), np.float16),
            }
            for _ in range(B)
        ]
        for run, fin in (runA, runB):
            fin(run(z))
        _CACHE["runners"] = (runA, runB)
    return _CACHE["runners"]


def kernel(x, Wq, Wk, Wv):
    x = np.asarray(x)
    (runA, finA), (runB, finB) = _get_runners()

    w16 = [np.asarray(w).astype(np.float16) for w in (Wq, Wk, Wv)]
    mapsA = [
        {"xT": np.ascontiguousarray(x[b].T.astype(np.float16)),
         "Wq": w16[0], "Wk": w16[1], "Wv": w16[2]}
        for b in range(B)
    ]
    mapsB = [dict(m) for m in mapsA]
    # dispatch both meshes before blocking on either
    outA = runA(mapsA)
    outB = runB(mapsB)
    resA = finA(outA)
    resB = finB(outB)

    full = np.empty((B, T, H), np.float32)
    for b in range(B):
        oa, ob = resA[b]["out"], resB[b]["out"]
        for j, t0 in enumerate(T0S_A):
            full[b, t0:t0 + TQ] = oa[j * TQ:(j + 1) * TQ]
        for j, t0 in enumerate(T0S_B):
            full[b, t0:t0 + TQ] = ob[j * TQ:(j + 1) * TQ]
    return full


# revision 22
# speedup vs baseline: 1.1018x; 1.1018x over previous
"""Single-head causal attention on 8 Trainium2 NeuronCores.

B=4, T=4096, E=1024, H=128, fp32 in/out.

Sharding: batch-parallel x query-parallel. Two programs (one per query half):
  program A cores (devices 0-3): batch d, queries [0:1024) u [3072:4096)
  program B cores (devices 4-7): batch d-4, queries [1024:3072)
Both halves have identical causal work (72 key-tiles of 128) -> balanced.
Each core computes full K/V for its batch from x^T (host-transposed, fp16).

On-chip (per core); matmul operands fp16, accumulation fp32:
  1. QKV projections over 8 E-chunks accumulated in PSUM.
     K^T [H=128p, T] fp16, V^T fp16, Q^T [128p, 2048] fp16.
     V natural via PE transpose of V^T.
  2. Per 512-query tile, key tiles of 128 (diagonal emitted first):
     S^T[keys,q] = (K^T_kt).T @ Q^T_j -> PSUM fp32,
     E = exp(S^T/sqrt(H)) on ScalarE -> fp16 SBUF,
     causal mask on diagonal tiles via gpsimd affine_select,
     PV: OT[h,q] += V_kt.T @ E in PSUM; G0/G1 += E on DVE (fp16 2x mode).
  3. denom = colsum(G0)+colsum(G1) via ones-matmuls into PSUM, DVE
     reciprocal, PE-transpose OT chunks, row-scale by 1/denom, one
     gathered DMA per query tile -> out natural [2048,128] fp32.
"""

import numpy as np

import concourse.bass as bass
import concourse.bacc as bacc
import concourse.mybir as mybir
import concourse.tile as tile
from concourse.masks import make_identity

B, T, E, H = 4, 4096, 1024, 128
TQ = 512          # query tile width
NE = E // 128     # 8 e-chunks
NCC = T // TQ     # 8 column chunks of T
QROWS = 2048      # queries per core
SCALE = float(H) ** -0.5
F32 = mybir.dt.float32
F16 = mybir.dt.float16

T0S_A = [0, 512, 3072, 3584]
T0S_B = [1024, 1536, 2048, 2560]


def _build(t0s):
    nc = bacc.Bacc("TRN2", target_bir_lowering=False, debug=False, num_devices=4)
    xT = nc.declare_dram_parameter("xT", [E, T], F16, isOutput=False)
    Wq = nc.declare_dram_parameter("Wq", [E, H], F16, isOutput=False)
    Wk = nc.declare_dram_parameter("Wk", [E, H], F16, isOutput=False)
    Wv = nc.declare_dram_parameter("Wv", [E, H], F16, isOutput=False)
    out = nc.declare_dram_parameter("out", [QROWS, H], F32, isOutput=True)

    qcc = {t0 // TQ: j for j, t0 in enumerate(t0s)}  # T col-chunk -> q tile slot

    with tile.TileContext(nc) as tc:
        with (
            tc.tile_pool(name="const", bufs=1) as const_pool,
            tc.tile_pool(name="wts", bufs=1) as wt_pool,
            tc.tile_pool(name="big", bufs=1) as big_pool,
            tc.tile_pool(name="ev", bufs=4) as e_pool,
            tc.tile_pool(name="g", bufs=2) as g_pool,
            tc.tile_pool(name="ot", bufs=2) as ot_pool,
            tc.tile_pool(name="small", bufs=4) as small_pool,
            tc.tile_pool(name="onat", bufs=2) as onat_pool,
            tc.tile_pool(name="mm512", bufs=3, space="PSUM") as mm_psum,
            tc.tile_pool(name="pv", bufs=2, space="PSUM") as pv_psum,
            tc.tile_pool(name="tp", bufs=2, space="PSUM") as tp_psum,
            tc.tile_pool(name="dcol", bufs=1, space="PSUM") as dcol_psum,
        ):
            ident16 = const_pool.tile([128, 128], F16, tag="id16")
            make_identity(nc, ident16[:])
            ident32 = const_pool.tile([128, 128], F32, tag="id32")
            make_identity(nc, ident32[:])
            ones = const_pool.tile([128, 1], F16, tag="ones")
            nc.gpsimd.memset(ones[:], 1.0)

            # weights: [E,H] dram -> [128, NE*H] sbuf, e-slice at [:, e*H:(e+1)*H]
            wq_sb = wt_pool.tile([128, NE * H], F16, tag="wq")
            wk_sb = wt_pool.tile([128, NE * H], F16, tag="wk")
            wv_sb = wt_pool.tile([128, NE * H], F16, tag="wv")
            for w_dram, w_sb in ((Wq, wq_sb), (Wk, wk_sb), (Wv, wv_sb)):
                nc.sync.dma_start(
                    out=w_sb[:].rearrange("p (e h) -> p e h", e=NE),
                    in_=w_dram[:].rearrange("(e p) h -> p e h", p=128),
                )

            # x^T resident, loaded in consumption order across three issue
            # engines. Program B only ever touches keys/queries < kv_cols.
            kv_ccs = max((t0 + TQ) // TQ for t0 in t0s)       # 8 for A, 6 for B
            kv_cols = kv_ccs * TQ
            xts = []
            for e in range(NE):
                xt_t = big_pool.tile([128, kv_cols], F16, tag=f"xt{e}")
                xts.append(xt_t)
            for e in range(NE):         # cols 0:512 on sync: PE starts fast
                nc.sync.dma_start(
                    out=xts[e][:, 0:TQ],
                    in_=xT[e * 128:(e + 1) * 128, 0:TQ],
                )
            for e in range(NE):         # cols 512:1024 on scalar (idle early)
                nc.scalar.dma_start(
                    out=xts[e][:, TQ:1024],
                    in_=xT[e * 128:(e + 1) * 128, TQ:1024],
                )
            for q in range(1, 3):       # cols 1024:3072 on sync, [128,1024]
                for e in range(NE):
                    nc.sync.dma_start(
                        out=xts[e][:, q * 1024:(q + 1) * 1024],
                        in_=xT[e * 128:(e + 1) * 128, q * 1024:(q + 1) * 1024],
                    )
            if kv_cols > 3072:          # cols 3072:4096 (A only), on sync;
                for e in range(NE):      # gpsimd stays free for causal masks
                    nc.sync.dma_start(
                        out=xts[e][:, 3072:kv_cols],
                        in_=xT[e * 128:(e + 1) * 128, 3072:kv_cols],
                    )

            KT = big_pool.tile([128, kv_cols], F16, tag="kt")   # K^T
            VT = big_pool.tile([128, kv_cols], F16, tag="vt")   # V^T
            V = big_pool.tile([128, kv_cols], F16, tag="v")     # V natural
            QT = big_pool.tile([128, QROWS], F16, tag="qt")     # Q^T

            def project(cc):
                c0 = cc * TQ
                for w_sb, dst, d0 in (
                    (wk_sb, KT, c0),
                    (wv_sb, VT, c0),
                ) + (((wq_sb, QT, qcc[cc] * TQ),) if cc in qcc else ()):
                    ps = mm_psum.tile([128, TQ], F32, tag="mm", name="ps")
                    for e in range(NE):
                        nc.tensor.matmul(
                            ps[:], w_sb[:, e * H:(e + 1) * H],
                            xts[e][:, c0:c0 + TQ],
                            start=(e == 0), stop=(e == NE - 1),
                        )
                    nc.vector.tensor_copy(dst[:, d0:d0 + TQ], ps[:])
                # V natural = transpose(V^T) for this chunk's key blocks
                for kt in range(cc * TQ // 128, (cc + 1) * TQ // 128):
                    tp16 = tp_psum.tile([128, 128], F16, tag="tp", name="tp16")
                    nc.tensor.transpose(
                        tp16[:], VT[:, kt * 128:(kt + 1) * 128], ident16[:]
                    )
                    nc.vector.tensor_copy(V[:, kt * 128:(kt + 1) * 128], tp16[:])

            def attend(j, t0):
                nkt = (t0 + TQ) // 128
                diag0 = t0 // 128
                ot_ps = pv_psum.tile([128, TQ], F32, name="ot_ps")
                G0 = g_pool.tile([128, TQ], F16, tag="g0", name="G0")
                G1 = g_pool.tile([128, TQ], F16, tag="g1", name="G1")
                kts = list(range(nkt - 1, -1, -1))  # diagonal first
                for i, kt in enumerate(kts):
                    st = mm_psum.tile([128, TQ], F32, tag="mm", name="st")
                    nc.tensor.matmul(
                        st[:], KT[:, kt * 128:(kt + 1) * 128],
                        QT[:, j * TQ:(j + 1) * TQ],
                        start=True, stop=True,
                    )
                    e_t = e_pool.tile([128, TQ], F16, name="e_t")
                    nc.scalar.activation(
                        e_t[:], st[:], mybir.ActivationFunctionType.Exp, scale=SCALE
                    )
                    if kt >= diag0:
                        # keep E[p,c] iff (t0+c) - (128*kt+p) >= 0
                        nc.gpsimd.affine_select(
                            out=e_t[:], in_=e_t[:],
                            compare_op=mybir.AluOpType.is_ge,
                            fill=0.0, base=t0 - 128 * kt,
                            pattern=[[1, TQ]], channel_multiplier=-1,
                        )
                    nc.tensor.matmul(
                        ot_ps[:], V[:, kt * 128:(kt + 1) * 128], e_t[:],
                        start=(i == 0), stop=(i == nkt - 1),
                    )
                    # exp-sum accumulation, two chains (halves dep depth)
                    if i < 2:
                        nc.vector.tensor_copy((G0 if i == 0 else G1)[:], e_t[:])
                    elif i % 2 == 0:
                        nc.vector.tensor_add(G0[:], G0[:], e_t[:])
                    else:
                        nc.vector.tensor_add(G1[:], G1[:], e_t[:])

                ot_sb = ot_pool.tile([128, TQ], F32, name="ot_sb")
                nc.scalar.copy(ot_sb[:], ot_ps[:])
                onat = onat_pool.tile([128, 4 * 128], F32, name="onat")
                single_g = nkt < 2
                for c in range(TQ // 128):
                    dps = dcol_psum.tile([128, 1], F32, name="dps")
                    nc.tensor.matmul(
                        dps[:], G0[:, c * 128:(c + 1) * 128], ones[:],
                        start=True, stop=single_g,
                    )
                    if not single_g:
                        nc.tensor.matmul(
                            dps[:], G1[:, c * 128:(c + 1) * 128], ones[:],
                            start=False, stop=True,
                        )
                    rc = small_pool.tile([128, 1], F32, name="rc")
                    nc.vector.reciprocal(rc[:], dps[:])
                    tp = tp_psum.tile([128, 128], F32, tag="tp", name="tp")
                    nc.tensor.transpose(
                        tp[:], ot_sb[:, c * 128:(c + 1) * 128], ident32[:]
                    )
                    nc.vector.tensor_scalar_mul(
                        onat[:, c * 128:(c + 1) * 128], tp[:], rc[:]
                    )
                # one DMA per query tile: [128, (c h)] -> out rows [j*TQ, +TQ)
                nc.sync.dma_start(
                    out=out[j * TQ:(j + 1) * TQ, :].rearrange(
                        "(c p) h -> p c h", p=128
                    ),
                    in_=onat[:].rearrange("p (c h) -> p c h", c=4),
                )

            # interleave: project each chunk, then run any attention tile
            # whose keys/queries are now fully projected
            done = set()
            for cc in range(kv_ccs):
                project(cc)
                for j, t0 in enumerate(t0s):
                    if j in done:
                        continue
                    # one chunk of slack after strictly-ready to avoid PE
                    # head-of-line stalls on exp/mask deps
                    if (t0 + TQ) // TQ <= cc and (t0 // TQ) < cc or cc == kv_ccs - 1:
                        done.add(j)
                        attend(j, t0)
            assert done == set(range(len(t0s)))

    nc.finalize()
    return nc


# ---------------- host-side run ----------------

_CACHE = {}


def _runner(nc, devices):
    """run_bass_via_pjrt with an explicit device list (subset launch)."""
    import jax
    from jax.sharding import Mesh, PartitionSpec
    from jax.experimental.shard_map import shard_map
    from concourse.bass2jax import _bass_exec_p, install_neuronx_cc_hook

    install_neuronx_cc_hook()
    n_cores = len(devices)
    part_name = nc.partition_id_tensor.name if nc.partition_id_tensor else None
    in_names, out_names, out_avals, zero_outs = [], [], [], []
    for alloc in nc.m.functions[0].allocations:
        if not isinstance(alloc, mybir.MemoryLocationSet):
            continue
        name = alloc.memorylocations[0].name
        if alloc.kind == "ExternalInput":
            if name != part_name:
                in_names.append(name)
        elif alloc.kind == "ExternalOutput":
            shape = tuple(alloc.tensor_shape)
            dtype = mybir.dt.np(alloc.dtype)
            out_names.append(name)
            out_avals.append(jax.core.ShapedArray(shape, dtype))
            zero_outs.append(np.zeros(shape, dtype))
    n_params = len(in_names)
    n_outs = len(out_avals)
    in_names = in_names + out_names
    if part_name is not None:
        in_names = in_names + [part_name]
    donate = tuple(range(n_params, n_params + n_outs))

    def _body(*args):
        from concourse.bass2jax import partition_id_tensor
        operands = list(args)
        if part_name is not None:
            operands.append(partition_id_tensor())
        outs = _bass_exec_p.bind(
            *operands,
            out_avals=tuple(out_avals),
            in_names=tuple(in_names),
            out_names=tuple(out_names),
            lowering_input_output_aliases=(),
            sim_require_finite=True,
            sim_require_nnan=True,
            nc=nc,
        )
        return tuple(outs)

    mesh = Mesh(np.asarray(devices), ("core",))
    sharded = jax.jit(
        shard_map(
            _body, mesh=mesh,
            in_specs=(PartitionSpec("core"),) * (n_params + n_outs),
            out_specs=(PartitionSpec("core"),) * n_outs,
            check_rep=False,
        ),
        donate_argnums=donate, keep_unused=True,
    )

    def run(in_maps):
        per_core = [[np.asarray(m[n]) for n in in_names[:n_params]] for m in in_maps]
        concat_in = [
            np.concatenate([per_core[c][i] for c in range(n_cores)], axis=0)
            for i in range(n_params)
        ]
        concat_zeros = [
            np.zeros((n_cores * z.shape[0], *z.shape[1:]), z.dtype) for z in zero_outs
        ]
        return sharded(*concat_in, *concat_zeros)

    def finish(out_arrs):
        return [
            {
                n: np.asarray(out_arrs[i]).reshape(n_cores, *out_avals[i].shape)[c]
                for i, n in enumerate(out_names)
            }
            for c in range(n_cores)
        ]

    return run, finish


def _get_runners():
    if "runners" not in _CACHE:
        import jax
        devs = jax.devices()
        ncA = _build(T0S_A)
        ncB = _build(T0S_B)
        _CACHE["ncs"] = (ncA, ncB)
        runA = _runner(ncA, devs[0:4])
        runB = _runner(ncB, devs[4:8])
        # Warm each executable once, sequentially and blocking, before
        # any concurrent use (cold concurrent dispatch has raced before).
        z = [
            {
                "xT": np.zeros((E, T), np.float16),
                "Wq": np.zeros((E, H), np.float16),
                "Wk": np.zeros((E, H), np.float16),
                "Wv": np.zeros((E, H), np.float16),
            }
            for _ in range(B)
        ]
        for run, fin in (runA, runB):
            fin(run(z))
        _CACHE["runners"] = (runA, runB)
    return _CACHE["runners"]


def kernel(x, Wq, Wk, Wv):
    x = np.asarray(x)
    (runA, finA), (runB, finB) = _get_runners()

    w16 = [np.asarray(w).astype(np.float16) for w in (Wq, Wk, Wv)]
    mapsA = [
        {"xT": np.ascontiguousarray(x[b].T.astype(np.float16)),
         "Wq": w16[0], "Wk": w16[1], "Wv": w16[2]}
        for b in range(B)
    ]
    mapsB = [dict(m) for m in mapsA]
    # dispatch both meshes before blocking on either
    outA = runA(mapsA)
    outB = runB(mapsB)
    resA = finA(outA)
    resB = finB(outB)

    full = np.empty((B, T, H), np.float32)
    for b in range(B):
        oa, ob = resA[b]["out"], resB[b]["out"]
        for j, t0 in enumerate(T0S_A):
            full[b, t0:t0 + TQ] = oa[j * TQ:(j + 1) * TQ]
        for j, t0 in enumerate(T0S_B):
            full[b, t0:t0 + TQ] = ob[j * TQ:(j + 1) * TQ]
    return full


# revision 23
# speedup vs baseline: 1.1059x; 1.0037x over previous
"""Single-head causal attention on 8 Trainium2 NeuronCores.

B=4, T=4096, E=1024, H=128, fp32 in/out.

Sharding: batch-parallel x query-parallel. Two programs (one per query half):
  program A cores (devices 0-3): batch d, queries [0:1024) u [3072:4096)
  program B cores (devices 4-7): batch d-4, queries [1024:3072)
Both halves have identical causal work (72 key-tiles of 128) -> balanced.
Each core computes full K/V for its batch from x^T (host-transposed, fp16).

On-chip (per core); matmul operands fp16, accumulation fp32:
  1. QKV projections over 8 E-chunks accumulated in PSUM.
     K^T [H=128p, T] fp16, V^T fp16, Q^T [128p, 2048] fp16.
     V natural via PE transpose of V^T.
  2. Per 512-query tile, key tiles of 128 (diagonal emitted first):
     S^T[keys,q] = (K^T_kt).T @ Q^T_j -> PSUM fp32,
     E = exp(S^T/sqrt(H)) on ScalarE -> fp16 SBUF,
     causal mask on diagonal tiles via gpsimd affine_select,
     PV: OT[h,q] += V_kt.T @ E in PSUM; G0/G1 += E on DVE (fp16 2x mode).
  3. denom = colsum(G0)+colsum(G1) via ones-matmuls into PSUM, DVE
     reciprocal, PE-transpose OT chunks, row-scale by 1/denom, one
     gathered DMA per query tile -> out natural [2048,128] fp32.
"""

import numpy as np

import concourse.bass as bass
import concourse.bacc as bacc
import concourse.mybir as mybir
import concourse.tile as tile
from concourse.masks import make_identity

B, T, E, H = 4, 4096, 1024, 128
TQ = 512          # query tile width
NE = E // 128     # 8 e-chunks
NCC = T // TQ     # 8 column chunks of T
QROWS = 2048      # queries per core
SCALE = float(H) ** -0.5
F32 = mybir.dt.float32
F16 = mybir.dt.float16

T0S_A = [0, 512, 2560, 3584]
T0S_B = [1024, 1536, 2048, 3072]


def _build(t0s):
    nc = bacc.Bacc("TRN2", target_bir_lowering=False, debug=False, num_devices=4)
    xT = nc.declare_dram_parameter("xT", [E, T], F16, isOutput=False)
    Wq = nc.declare_dram_parameter("Wq", [E, H], F16, isOutput=False)
    Wk = nc.declare_dram_parameter("Wk", [E, H], F16, isOutput=False)
    Wv = nc.declare_dram_parameter("Wv", [E, H], F16, isOutput=False)
    out = nc.declare_dram_parameter("out", [QROWS, H], F32, isOutput=True)

    qcc = {t0 // TQ: j for j, t0 in enumerate(t0s)}  # T col-chunk -> q tile slot

    with tile.TileContext(nc) as tc:
        with (
            tc.tile_pool(name="const", bufs=1) as const_pool,
            tc.tile_pool(name="wts", bufs=1) as wt_pool,
            tc.tile_pool(name="big", bufs=1) as big_pool,
            tc.tile_pool(name="ev", bufs=4) as e_pool,
            tc.tile_pool(name="g", bufs=2) as g_pool,
            tc.tile_pool(name="ot", bufs=2) as ot_pool,
            tc.tile_pool(name="small", bufs=4) as small_pool,
            tc.tile_pool(name="onat", bufs=2) as onat_pool,
            tc.tile_pool(name="mm512", bufs=3, space="PSUM") as mm_psum,
            tc.tile_pool(name="pv", bufs=2, space="PSUM") as pv_psum,
            tc.tile_pool(name="tp", bufs=2, space="PSUM") as tp_psum,
            tc.tile_pool(name="dcol", bufs=1, space="PSUM") as dcol_psum,
        ):
            ident16 = const_pool.tile([128, 128], F16, tag="id16")
            make_identity(nc, ident16[:])
            ident32 = const_pool.tile([128, 128], F32, tag="id32")
            make_identity(nc, ident32[:])
            ones = const_pool.tile([128, 1], F16, tag="ones")
            nc.gpsimd.memset(ones[:], 1.0)

            # weights: [E,H] dram -> [128, NE*H] sbuf, e-slice at [:, e*H:(e+1)*H]
            wq_sb = wt_pool.tile([128, NE * H], F16, tag="wq")
            wk_sb = wt_pool.tile([128, NE * H], F16, tag="wk")
            wv_sb = wt_pool.tile([128, NE * H], F16, tag="wv")
            for w_dram, w_sb in ((Wq, wq_sb), (Wk, wk_sb), (Wv, wv_sb)):
                nc.sync.dma_start(
                    out=w_sb[:].rearrange("p (e h) -> p e h", e=NE),
                    in_=w_dram[:].rearrange("(e p) h -> p e h", p=128),
                )

            # x^T: one e-major DMA per 512-col tier -> [128, e*TQ+c].
            # Single issue + 1 MiB wire per tier; tiers land progressively
            # in consumption order.
            kv_ccs = max((t0 + TQ) // TQ for t0 in t0s)       # 8 for A, 7 for B
            kv_cols = kv_ccs * TQ
            xts = []
            for cc in range(kv_ccs):
                xt_t = big_pool.tile([128, NE * TQ], F16, tag=f"xt{cc}")
                nc.sync.dma_start(
                    out=xt_t[:].rearrange("p (e c) -> p e c", e=NE),
                    in_=xT[:, cc * TQ:(cc + 1) * TQ].rearrange(
                        "(e p) c -> p e c", p=128
                    ),
                )
                xts.append(xt_t)

            KT = big_pool.tile([128, kv_cols], F16, tag="kt")   # K^T
            VT = big_pool.tile([128, kv_cols], F16, tag="vt")   # V^T
            V = big_pool.tile([128, kv_cols], F16, tag="v")     # V natural
            QT = big_pool.tile([128, QROWS], F16, tag="qt")     # Q^T

            def project(cc):
                c0 = cc * TQ
                for w_sb, dst, d0 in (
                    (wk_sb, KT, c0),
                    (wv_sb, VT, c0),
                ) + (((wq_sb, QT, qcc[cc] * TQ),) if cc in qcc else ()):
                    ps = mm_psum.tile([128, TQ], F32, tag="mm", name="ps")
                    for e in range(NE):
                        nc.tensor.matmul(
                            ps[:], w_sb[:, e * H:(e + 1) * H],
                            xts[cc][:, e * TQ:(e + 1) * TQ],
                            start=(e == 0), stop=(e == NE - 1),
                        )
                    nc.vector.tensor_copy(dst[:, d0:d0 + TQ], ps[:])
                # V natural = transpose(V^T) for this chunk's key blocks
                for kt in range(cc * TQ // 128, (cc + 1) * TQ // 128):
                    tp16 = tp_psum.tile([128, 128], F16, tag="tp", name="tp16")
                    nc.tensor.transpose(
                        tp16[:], VT[:, kt * 128:(kt + 1) * 128], ident16[:]
                    )
                    nc.vector.tensor_copy(V[:, kt * 128:(kt + 1) * 128], tp16[:])

            def attend(j, t0):
                nkt = (t0 + TQ) // 128
                diag0 = t0 // 128
                ot_ps = pv_psum.tile([128, TQ], F32, name="ot_ps")
                G0 = g_pool.tile([128, TQ], F16, tag="g0", name="G0")
                G1 = g_pool.tile([128, TQ], F16, tag="g1", name="G1")
                kts = list(range(nkt - 1, -1, -1))  # diagonal first
                for i, kt in enumerate(kts):
                    st = mm_psum.tile([128, TQ], F32, tag="mm", name="st")
                    nc.tensor.matmul(
                        st[:], KT[:, kt * 128:(kt + 1) * 128],
                        QT[:, j * TQ:(j + 1) * TQ],
                        start=True, stop=True,
                    )
                    e_t = e_pool.tile([128, TQ], F16, name="e_t")
                    nc.scalar.activation(
                        e_t[:], st[:], mybir.ActivationFunctionType.Exp, scale=SCALE
                    )
                    if kt >= diag0:
                        # keep E[p,c] iff (t0+c) - (128*kt+p) >= 0
                        nc.gpsimd.affine_select(
                            out=e_t[:], in_=e_t[:],
                            compare_op=mybir.AluOpType.is_ge,
                            fill=0.0, base=t0 - 128 * kt,
                            pattern=[[1, TQ]], channel_multiplier=-1,
                        )
                    nc.tensor.matmul(
                        ot_ps[:], V[:, kt * 128:(kt + 1) * 128], e_t[:],
                        start=(i == 0), stop=(i == nkt - 1),
                    )
                    # exp-sum accumulation, two chains (halves dep depth)
                    if i < 2:
                        nc.vector.tensor_copy((G0 if i == 0 else G1)[:], e_t[:])
                    elif i % 2 == 0:
                        nc.vector.tensor_add(G0[:], G0[:], e_t[:])
                    else:
                        nc.vector.tensor_add(G1[:], G1[:], e_t[:])

                ot_sb = ot_pool.tile([128, TQ], F32, name="ot_sb")
                nc.scalar.copy(ot_sb[:], ot_ps[:])
                onat = onat_pool.tile([128, 4 * 128], F32, name="onat")
                single_g = nkt < 2
                for c in range(TQ // 128):
                    dps = dcol_psum.tile([128, 1], F32, name="dps")
                    nc.tensor.matmul(
                        dps[:], G0[:, c * 128:(c + 1) * 128], ones[:],
                        start=True, stop=single_g,
                    )
                    if not single_g:
                        nc.tensor.matmul(
                            dps[:], G1[:, c * 128:(c + 1) * 128], ones[:],
                            start=False, stop=True,
                        )
                    rc = small_pool.tile([128, 1], F32, name="rc")
                    nc.vector.reciprocal(rc[:], dps[:])
                    tp = tp_psum.tile([128, 128], F32, tag="tp", name="tp")
                    nc.tensor.transpose(
                        tp[:], ot_sb[:, c * 128:(c + 1) * 128], ident32[:]
                    )
                    nc.vector.tensor_scalar_mul(
                        onat[:, c * 128:(c + 1) * 128], tp[:], rc[:]
                    )
                # one DMA per query tile: [128, (c h)] -> out rows [j*TQ, +TQ)
                nc.sync.dma_start(
                    out=out[j * TQ:(j + 1) * TQ, :].rearrange(
                        "(c p) h -> p c h", p=128
                    ),
                    in_=onat[:].rearrange("p (c h) -> p c h", c=4),
                )

            # interleave: project each chunk, then run any attention tile
            # whose keys/queries are now fully projected
            done = set()
            for cc in range(kv_ccs):
                project(cc)
                for j, t0 in enumerate(t0s):
                    if j in done:
                        continue
                    # one chunk of slack after strictly-ready to avoid PE
                    # head-of-line stalls on exp/mask deps
                    if (t0 + TQ) // TQ <= cc and (t0 // TQ) < cc or cc == kv_ccs - 1:
                        done.add(j)
                        attend(j, t0)
            assert done == set(range(len(t0s)))

    nc.finalize()
    return nc


# ---------------- host-side run ----------------

_CACHE = {}


def _runner(nc, devices):
    """run_bass_via_pjrt with an explicit device list (subset launch)."""
    import jax
    from jax.sharding import Mesh, PartitionSpec
    from jax.experimental.shard_map import shard_map
    from concourse.bass2jax import _bass_exec_p, install_neuronx_cc_hook

    install_neuronx_cc_hook()
    n_cores = len(devices)
    part_name = nc.partition_id_tensor.name if nc.partition_id_tensor else None
    in_names, out_names, out_avals, zero_outs = [], [], [], []
    for alloc in nc.m.functions[0].allocations:
        if not isinstance(alloc, mybir.MemoryLocationSet):
            continue
        name = alloc.memorylocations[0].name
        if alloc.kind == "ExternalInput":
            if name != part_name:
                in_names.append(name)
        elif alloc.kind == "ExternalOutput":
            shape = tuple(alloc.tensor_shape)
            dtype = mybir.dt.np(alloc.dtype)
            out_names.append(name)
            out_avals.append(jax.core.ShapedArray(shape, dtype))
            zero_outs.append(np.zeros(shape, dtype))
    n_params = len(in_names)
    n_outs = len(out_avals)
    in_names = in_names + out_names
    if part_name is not None:
        in_names = in_names + [part_name]
    donate = tuple(range(n_params, n_params + n_outs))

    def _body(*args):
        from concourse.bass2jax import partition_id_tensor
        operands = list(args)
        if part_name is not None:
            operands.append(partition_id_tensor())
        outs = _bass_exec_p.bind(
            *operands,
            out_avals=tuple(out_avals),
            in_names=tuple(in_names),
            out_names=tuple(out_names),
            lowering_input_output_aliases=(),
            sim_require_finite=True,
            sim_require_nnan=True,
            nc=nc,
        )
        return tuple(outs)

    mesh = Mesh(np.asarray(devices), ("core",))
    sharded = jax.jit(
        shard_map(
            _body, mesh=mesh,
            in_specs=(PartitionSpec("core"),) * (n_params + n_outs),
            out_specs=(PartitionSpec("core"),) * n_outs,
            check_rep=False,
        ),
        donate_argnums=donate, keep_unused=True,
    )

    def run(in_maps):
        per_core = [[np.asarray(m[n]) for n in in_names[:n_params]] for m in in_maps]
        concat_in = [
            np.concatenate([per_core[c][i] for c in range(n_cores)], axis=0)
            for i in range(n_params)
        ]
        concat_zeros = [
            np.zeros((n_cores * z.shape[0], *z.shape[1:]), z.dtype) for z in zero_outs
        ]
        return sharded(*concat_in, *concat_zeros)

    def finish(out_arrs):
        return [
            {
                n: np.asarray(out_arrs[i]).reshape(n_cores, *out_avals[i].shape)[c]
                for i, n in enumerate(out_names)
            }
            for c in range(n_cores)
        ]

    return run, finish


def _get_runners():
    if "runners" not in _CACHE:
        import jax
        devs = jax.devices()
        ncA = _build(T0S_A)
        ncB = _build(T0S_B)
        _CACHE["ncs"] = (ncA, ncB)
        runA = _runner(ncA, devs[0:4])
        runB = _runner(ncB, devs[4:8])
        # Warm each executable once, sequentially and blocking, before
        # any concurrent use (cold concurrent dispatch has raced before).
        z = [
            {
                "xT": np.zeros((E, T), np.float16),
                "Wq": np.zeros((E, H), np.float16),
                "Wk": np.zeros((E, H), np.float16),
                "Wv": np.zeros((E, H), np.float16),
            }
            for _ in range(B)
        ]
        for run, fin in (runA, runB):
            fin(run(z))
        _CACHE["runners"] = (runA, runB)
    return _CACHE["runners"]


def kernel(x, Wq, Wk, Wv):
    x = np.asarray(x)
    (runA, finA), (runB, finB) = _get_runners()

    w16 = [np.asarray(w).astype(np.float16) for w in (Wq, Wk, Wv)]
    mapsA = [
        {"xT": np.ascontiguousarray(x[b].T.astype(np.float16)),
         "Wq": w16[0], "Wk": w16[1], "Wv": w16[2]}
        for b in range(B)
    ]
    mapsB = [dict(m) for m in mapsA]
    # dispatch both meshes before blocking on either
    outA = runA(mapsA)
    outB = runB(mapsB)
    resA = finA(outA)
    resB = finB(outB)

    full = np.empty((B, T, H), np.float32)
    for b in range(B):
        oa, ob = resA[b]["out"], resB[b]["out"]
        for j, t0 in enumerate(T0S_A):
            full[b, t0:t0 + TQ] = oa[j * TQ:(j + 1) * TQ]
        for j, t0 in enumerate(T0S_B):
            full[b, t0:t0 + TQ] = ob[j * TQ:(j + 1) * TQ]
    return full
